# revision 21
# baseline (speedup 1.0000x reference)
"""Trainium2 Bass kernel for nn_Attention_54391465836966.

The reference's .reshape calls are RAW byte reinterpretations: token matrix
T = content_feat[b] bytes viewed [S, C] (not a transpose), and s (token-major
[S, C]) is viewed [C, S] before the 1x1 conv.  The host passes every input
pre-arranged into its exact SBUF image (one [128, X] contiguous DMA each),
with the token views pre-transposed to channel-major, so the device does no
PE transposes; the s view is realized with SBUF->SBUF DMAs that re-pair
token rows (s2d[r] = tokens (2r, 2r+1) concatenated).

Per core (b = core//4, n = core%4), channel-major [C, S] throughout:
  ctok = cfT + posT ; ctmp = compT + posT
  qT = Wq^T ctok ; kT = Wkv[:, :C]^T ctmp ; v = ctmp^T Wkv[:, C:]
  per head h: P = exp(scale k_h^T q); o_h = (v_h^T P) / Z   (Z via ones col)
  s_tok = packed^T Wproj                                     (token-major)
  const (token-quarter n, full scale): s_cq = ctokQ^T Wproj + bproj
  out_p = WconvT[:C]^T s2d + WconvT[quarter]^T s2d_cq + bconv/4
  out_cf = WconvT[C:, out-quarter]^T cf_raw                  (host-placed)
Host sums the 4 component partials per batch.  The affine const terms are
distributed: each core owns token-quarter n of the ctok-proj path and
out-channel quarter n of the cf-conv, plus bconv/4 — no gated-zero work.
Heads are packed two-per-tile so proj runs at 128 contraction.  P/V run in
bf16 (PE rate unchanged, half the SBUF traffic); other matmuls fp32r.
exp runs per kt-PAIR ([128, 2048]) to halve ACT access-latency overhead.
"""
import sys

sys.path.insert(0, "/opt/trn_rl_repo")

import numpy as np

N_CORES = 8
B, C, H, W = 2, 512, 32, 32
S = H * W  # 1024
NH, HD = 8, 64
SCALE = HD ** -0.5

_CACHE = {}


def _img(x, cols):
    """[512, cols] matrix -> its [128, 4*cols] SBUF image (4 row-blocks
    side by side)."""
    return np.ascontiguousarray(
        x.reshape(4, 128, cols).transpose(1, 0, 2).reshape(128, 4 * cols),
        dtype=np.float32)


def _build():
    if "nc" in _CACHE:
        return _CACHE["nc"]
    from contextlib import ExitStack

    import concourse.bacc as bacc
    import concourse.mybir as mybir
    import concourse.tile as tile

    f32 = mybir.dt.float32
    f32r = mybir.dt.float32r
    bf16 = mybir.dt.bfloat16
    EXP = mybir.ActivationFunctionType.Exp

    nc = bacc.Bacc("TRN2", target_bir_lowering=False, debug=False,
                   num_devices=N_CORES)

    din = lambda n, s: nc.dram_tensor(n, s, mybir.dt.float32r,
                                      kind="ExternalInput").ap()
    # all inputs are SBUF images: [128, 4*cols] with 4 row-blocks packed
    pos_d = din("pos", [128, 4096])      # posT image
    cmp_d = din("cmp", [128, 4096])      # compT image
    cft_d = din("cft", [128, 4096])      # cfT image
    cfr_d = din("cfr", [128, 4096])      # raw content_feat[b] image
    wk_d = din("wk", [128, 2048])        # Wkv[:, :C] image
    wv_d = din("wv", [128, 2048])        # Wkv[:, C:] image
    wq_d = din("wq", [128, 2048])        # Wq image
    wproj_d = din("wproj", [128, 2048])  # Wproj image
    wcvs_d = din("wcvs", [128, 2048])    # WconvT[:C] image
    wcvcq_d = din("wcvcq", [128, 512])   # WconvT[C:, out-quarter] image
    wcvsq_d = din("wcvsq", [128, 512])   # WconvT[128n:128(n+1), :]
    cftq_d = din("cftq", [128, 1024])    # cfT[:, token-quarter] image
    posq_d = din("posq", [128, 1024])    # posT[:, token-quarter] image
    bias_d = din("bias2", [1, 1024])     # [bproj, bconv/4]
    out_p = nc.dram_tensor("out_p", [C, S], f32, kind="ExternalOutput").ap()
    out_cf = nc.dram_tensor("out_cf", [128, S], f32,
                            kind="ExternalOutput").ap()

    with tile.TileContext(nc) as tc, ExitStack() as ctx:
        main = ctx.enter_context(tc.tile_pool(name="main", bufs=1))

        ones32 = main.tile([1, 512], f32, tag="ones32")
        nc.gpsimd.memset(ones32[:], 1.0)
        ones = main.tile([1, 512], f32r, tag="ones")
        nc.vector.tensor_copy(ones[:], ones32[:])

        # ---- front-critical DMAs: v/k path first ----
        pos_sb = [main.tile([128, 1024], f32r, tag=f"pos{j}", name=f"pos{j}")
                  for j in range(4)]
        cmp_sb = [main.tile([128, 1024], f32r, tag=f"cmp{j}", name=f"cmp{j}")
                  for j in range(4)]
        cft_sb = [main.tile([128, 1024], f32r, tag=f"cft{j}", name=f"cft{j}")
                  for j in range(4)]
        wv_sb = main.tile([128, 2048], f32r, tag="wv")
        wk_sb = main.tile([128, 2048], f32r, tag="wk")
        wq_sb = main.tile([128, 2048], f32r, tag="wq")
        ctok = [main.tile([128, S], f32r, tag=f"ctk{j}", name=f"ctok{j}")
                for j in range(4)]
        ctmp = [main.tile([128, S], f32r, tag=f"ct{j}", name=f"ctmp{j}")
                for j in range(4)]
        for j in range(4):
            nc.sync.dma_start(pos_sb[j][:], pos_d[:, 1024 * j:1024 * (j + 1)])
            nc.sync.dma_start(cmp_sb[j][:], cmp_d[:, 1024 * j:1024 * (j + 1)])
            if j == 0:
                nc.sync.dma_start(wv_sb[:, 0:1024], wv_d[:, 0:1024])
            if j == 1:
                nc.sync.dma_start(wv_sb[:, 1024:2048], wv_d[:, 1024:2048])
            if j == 2:
                nc.sync.dma_start(wk_sb[:, 0:1024], wk_d[:, 0:1024])
            if j == 3:
                nc.sync.dma_start(wk_sb[:, 1024:2048], wk_d[:, 1024:2048])
            nc.gpsimd.tensor_add(ctmp[j][:], cmp_sb[j][:], pos_sb[j][:])
        for j in range(4):
            nc.sync.dma_start(cft_sb[j][:], cft_d[:, 1024 * j:1024 * (j + 1)])
            nc.vector.tensor_add(ctok[j][:], cft_sb[j][:], pos_sb[j][:])
        nc.sync.dma_start(wq_sb[:], wq_d[:])

        # ---- late weights / const-path inputs ----
        cfr_sb = [main.tile([128, 1024], f32r, tag=f"cfr{j}", name=f"cfr{j}")
                  for j in range(4)]
        wproj_sb = main.tile([128, 2048], f32r, tag="wp")
        wcvs_sb = main.tile([128, 2048], f32r, tag="wcs")
        wcvcq_sb = main.tile([128, 512], f32r, tag="wcc")
        wcvsq_sb = main.tile([128, 512], f32r, tag="wcsq")
        cftq_sb = main.tile([128, 1024], f32r, tag="cftq")
        posq_sb = main.tile([128, 1024], f32r, tag="posq")
        bias_sb = main.tile([1, 1024], f32r, tag="bias")
        for j in range(4):
            nc.sync.dma_start(cfr_sb[j][:], cfr_d[:, 1024 * j:1024 * (j + 1)])
        nc.sync.dma_start(wcvcq_sb[:], wcvcq_d[:])
        nc.sync.dma_start(cftq_sb[:], cftq_d[:])
        nc.sync.dma_start(posq_sb[:], posq_d[:])
        nc.sync.dma_start(bias_sb[:], bias_d[:])
        nc.sync.dma_start(wproj_sb[:], wproj_d[:])
        nc.sync.dma_start(wcvs_sb[:], wcvs_d[:])
        nc.sync.dma_start(wcvsq_sb[:], wcvsq_d[:])

        # norm scratch + late tiles (several ride dead early slots)
        zraw = main.tile([1, S], f32, tag="zraw")
        zs2 = main.tile([1, S], f32, tag="zs2")
        zinv = main.tile([1, S], f32, tag="zinv")
        zbc = main.tile([128, S], f32, tag="zbc")
        ocf_sb = main.tile([128, S], f32, tag="cfr0", name="ocf")
        ctokq = main.tile([128, 1024], f32r, tag="wq", name="ctokq")
        packed = [main.tile([128, S], f32r,
                            tag=(f"cfr{j}" if j else "zzpk0"),
                            name=f"pk{j}") for j in range(4)]
        outp = [main.tile([128, S], f32, tag=f"ctk{j}", name=f"op{j}")
                for j in range(4)]

        with tc.tile_pool(name="psA", bufs=2, space="PSUM") as ps:
            # ---- v: [ki, hd] blocks per (kt, head) + ones col, bf16 ----
            v_sb = [main.tile([128, 1040], bf16, tag=f"v{t}", name=f"v{t}")
                    for t in range(4)]
            for t in range(4):
                nc.gpsimd.memset(v_sb[t][:], 1.0)
            for kt in range(8):
                acc = ps.tile([128, 512], f32, tag="mm")
                for k in range(4):
                    nc.tensor.matmul(acc[:],
                                     ctmp[k][:, 128 * kt:128 * (kt + 1)],
                                     wv_sb[:, 512 * k:512 * (k + 1)],
                                     start=(k == 0), stop=(k == 3))
                dst = v_sb[kt // 2][:, 520 * (kt % 2):520 * (kt % 2) + 520]
                nc.scalar.copy(
                    dst.rearrange("p (m c) -> p m c", m=8)[:, :, 0:64],
                    acc[:].rearrange("p (m c) -> p m c", m=8))

            # ---- kT / qT (ride cft/cmp slots) ----
            kT = [main.tile([128, S], f32r, tag=f"cft{j}", name=f"kT{j}")
                  for j in range(4)]
            qT = [main.tile([128, S], f32r, tag=f"cmp{j}", name=f"qT{j}")
                  for j in range(4)]
            for j in range(4):
                for qc in range(2):
                    acc = ps.tile([128, 512], f32, tag="mm")
                    for k in range(4):
                        nc.tensor.matmul(
                            acc[:],
                            wk_sb[:, 512 * k + 128 * j:512 * k + 128 * (j + 1)],
                            ctmp[k][:, 512 * qc:512 * (qc + 1)],
                            start=(k == 0), stop=(k == 3))
                    nc.vector.tensor_copy(kT[j][:, 512 * qc:512 * (qc + 1)],
                                          acc[:])
                    acc2 = ps.tile([128, 512], f32, tag="mm")
                    for k in range(4):
                        nc.tensor.matmul(
                            acc2[:],
                            wq_sb[:, 512 * k + 128 * j:512 * k + 128 * (j + 1)],
                            ctok[k][:, 512 * qc:512 * (qc + 1)],
                            start=(k == 0), stop=(k == 3))
                    nc.scalar.copy(qT[j][:, 512 * qc:512 * (qc + 1)],
                                   acc2[:])

            # ---- out_cf: cf-conv output-channel quarter (independent) ----
            for half in range(2):
                acc = ps.tile([128, 512], f32, tag="mm")
                for k in range(4):
                    nc.tensor.matmul(
                        acc[:], wcvcq_sb[:, 128 * k:128 * (k + 1)],
                        cfr_sb[k][:, 512 * half:512 * (half + 1)],
                        start=(k == 0), stop=(k == 3))
                nc.scalar.copy(ocf_sb[:, 512 * half:512 * (half + 1)], acc[:])
            nc.sync.dma_start(out_cf[:, :], ocf_sb[:])

            # ---- const pipeline: token-quarter proj + its s2d view ----
            nc.vector.tensor_add(ctokq[:], cftq_sb[:], posq_sb[:])
            s_cq = [main.tile([128, 512], f32r, tag=f"ct{i}", name=f"scq{i}")
                    for i in range(2)]
            s2d_cq = main.tile([128, S], f32r, tag="ct2", name="s2dcq")
            for i in range(2):
                acc = ps.tile([128, 512], f32, tag="mm")
                nc.tensor.matmul(acc[:], ones[0:1, 0:128],
                                 bias_sb[0:1, 0:512], start=True, stop=False)
                for a in range(4):
                    nc.tensor.matmul(
                        acc[:],
                        ctokq[:, 256 * a + 128 * i:256 * a + 128 * (i + 1)],
                        wproj_sb[:, 512 * a:512 * (a + 1)],
                        start=False, stop=(a == 3))
                nc.vector.tensor_copy(s_cq[i][:], acc[:])
            for g in range(2):
                for sh in range(2):
                    nc.sync.dma_start(
                        s2d_cq[64 * sh:64 * sh + 64, 512 * g:512 * g + 512],
                        s_cq[sh][g:128:2, :])

        # ---- attention + tail in one PSUM pool set ----
        with tc.tile_pool(name="psS", bufs=1, space="PSUM") as psS, \
             tc.tile_pool(name="psO", bufs=1, space="PSUM") as psO, \
             tc.tile_pool(name="psT", bufs=2, space="PSUM") as psT:
            # P tiles per kt-pair, bf16, riding the dead pos slots
            ptB = [main.tile([128, 2048], bf16, tag=f"pos{p}", name=f"ptB{p}")
                   for p in range(4)]
            for h in range(NH):
                j, row = h // 2, 64 * (h % 2)
                o_ps = psO.tile([65, S], f32, tag="o")
                for p in range(4):
                    sc = psS.tile([128, 2048], f32, tag="sc")
                    for par in range(2):
                        kt = 2 * p + par
                        for qc in range(2):
                            nc.tensor.matmul(
                                sc[:, 1024 * par + 512 * qc:
                                   1024 * par + 512 * (qc + 1)],
                                kT[j][row:row + 64, 128 * kt:128 * (kt + 1)],
                                qT[j][row:row + 64, 512 * qc:512 * (qc + 1)],
                                start=True, stop=True)
                    nc.scalar.activation(ptB[p][:], sc[:], EXP, scale=SCALE)
                    for par in range(2):
                        kt = 2 * p + par
                        vsl = v_sb[p][:, 520 * par + 65 * h:
                                      520 * par + 65 * h + 65]
                        for qc in range(2):
                            nc.tensor.matmul(
                                o_ps[:, 512 * qc:512 * (qc + 1)], vsl,
                                ptB[p][:, 1024 * par + 512 * qc:
                                       1024 * par + 512 * (qc + 1)],
                                start=(kt == 0), stop=(kt == 7))
                # normalization: Z row -> recip -> broadcast -> scale
                nc.vector.tensor_copy(zraw[0:1, :], o_ps[64:65, :])
                nc.vector.reciprocal_approx_accurate(
                    zinv[0:1, :], zraw[0:1, :], zs2[0:1, :])
                nc.gpsimd.partition_broadcast(zbc[:], zinv[0:1, :])
                rows = slice(64 * (h % 2), 64 * (h % 2) + 64)
                nc.vector.tensor_copy(packed[j][rows, :], o_ps[0:64, :])
                nc.gpsimd.tensor_mul(packed[j][rows, :],
                                     packed[j][rows, :], zbc[rows, :])

            # ---- proj (token-major, 4 mms/tile) -> s2d re-pair -> conv ----
            s_sb = [main.tile([128, 512], f32r, tag=f"cmp{i % 4}",
                              name=f"s{i}") for i in range(8)]
            s2d = [main.tile([128, S], f32r, tag=f"cft{i}", name=f"s2d{i}")
                   for i in range(4)]
            for i in range(8):
                acc = psT.tile([128, 512], f32, tag="pj")
                for jj in range(4):
                    nc.tensor.matmul(acc[:],
                                     packed[jj][:, 128 * i:128 * (i + 1)],
                                     wproj_sb[:, 512 * jj:512 * (jj + 1)],
                                     start=(jj == 0), stop=(jj == 3))
                eng = nc.vector if i % 2 == 0 else nc.scalar
                if eng is nc.scalar:
                    eng.copy(s_sb[i][:], acc[:])
                else:
                    eng.tensor_copy(s_sb[i][:], acc[:])
                if i % 2 == 1:
                    ii = i // 2
                    for g in range(2):
                        for sh in range(2):
                            src = s_sb[2 * ii + sh]
                            nc.sync.dma_start(
                                s2d[ii][64 * sh:64 * sh + 64,
                                        512 * g:512 * g + 512],
                                src[g:128:2, :])
            for oc in range(4):
                for half in range(2):
                    acc = psT.tile([128, 512], f32, tag="pj")
                    nc.tensor.matmul(
                        acc[:],
                        bias_sb[0:1, 512 + 128 * oc:512 + 128 * (oc + 1)],
                        ones[0:1, :], start=True, stop=False)
                    for r in range(4):
                        nc.tensor.matmul(
                            acc[:],
                            wcvs_sb[:, 512 * r + 128 * oc:
                                    512 * r + 128 * (oc + 1)],
                            s2d[r][:, 512 * half:512 * (half + 1)],
                            start=False, stop=False)
                    nc.tensor.matmul(
                        acc[:], wcvsq_sb[:, 128 * oc:128 * (oc + 1)],
                        s2d_cq[:, 512 * half:512 * (half + 1)],
                        start=False, stop=True)
                    eng = nc.vector if (oc + half) % 2 == 0 else nc.scalar
                    if eng is nc.scalar:
                        eng.copy(outp[oc][:, 512 * half:512 * (half + 1)],
                                 acc[:])
                    else:
                        eng.tensor_copy(
                            outp[oc][:, 512 * half:512 * (half + 1)], acc[:])
                    nc.sync.dma_start(
                        out_p[128 * oc:128 * (oc + 1),
                              512 * half:512 * (half + 1)],
                        outp[oc][:, 512 * half:512 * (half + 1)])

    nc.compile()
    _CACHE["nc"] = nc
    return nc


def _shard_inputs(content_feat, components, pos_emb, Wq, Wkv, Wproj, bproj,
                  Wconv, bconv):
    f = np.float32
    posT = pos_emb.reshape(S, C).T.astype(f)
    pos_img = _img(posT, S)
    wconvT = Wconv.T.astype(f)                        # [2C, C]
    wk_img = _img(np.ascontiguousarray(Wkv[:, :C]), C)
    wv_img = _img(np.ascontiguousarray(Wkv[:, C:]), C)
    wq_img = _img(Wq, C)
    wproj_img = _img(Wproj, C)
    wcvs_img = _img(np.ascontiguousarray(wconvT[:C]), C)
    in_maps = []
    for core in range(N_CORES):
        b, n = core // 4, core % 4
        sl = slice(128 * n, 128 * (n + 1))
        tq = slice(256 * n, 256 * (n + 1))
        cfT = np.ascontiguousarray(content_feat[b].reshape(S, C).T)
        bias2 = np.concatenate([bproj, bconv / 4]).reshape(1, 1024)
        in_maps.append({
            "pos": pos_img,
            "cmp": _img(components[n, b].reshape(S, C).T, S),
            "cft": _img(cfT, S),
            "cfr": _img(content_feat[b].reshape(C, S), S),
            "wk": wk_img,
            "wv": wv_img,
            "wq": wq_img,
            "wproj": wproj_img,
            "wcvs": wcvs_img,
            "wcvcq": _img(np.ascontiguousarray(wconvT[C:, sl]), 128),
            "wcvsq": np.ascontiguousarray(wconvT[sl], dtype=f),
            "cftq": _img(np.ascontiguousarray(cfT[:, tq]), 256),
            "posq": _img(np.ascontiguousarray(posT[:, tq]), 256),
            "bias2": np.ascontiguousarray(bias2, dtype=f),
        })
    return in_maps


def _run(trace=False, **inputs):
    from concourse.bass_utils import run_bass_kernel_spmd

    nc = _build()
    in_maps = _shard_inputs(**inputs)
    res = run_bass_kernel_spmd(nc, in_maps, list(range(N_CORES)), trace=trace)
    full = np.empty((B, C, S), dtype=np.float32)
    for b in range(B):
        acc = sum(res.results[4 * b + n]["out_p"] for n in range(4))
        for n in range(4):
            acc[128 * n:128 * (n + 1)] += res.results[4 * b + n]["out_cf"]
        full[b] = acc
    return full.reshape(B, C, H, W).astype(np.float32), res


def kernel(**inputs):
    out, _ = _run(trace=False, **inputs)
    return out


# revision 22
# speedup vs baseline: 1.2818x; 1.2818x over previous
"""Trainium2 Bass kernel for nn_Attention_54391465836966.

The reference's .reshape calls are RAW byte reinterpretations: token matrix
T = content_feat[b] bytes viewed [S, C] (not a transpose), and s (token-major
[S, C]) is viewed [C, S] before the 1x1 conv.  The host passes every input
pre-arranged into its exact SBUF image (one [128, X] contiguous DMA each),
with the token views pre-transposed to channel-major, so the device does no
PE transposes; the s view is realized with SBUF->SBUF DMAs that re-pair
token rows (s2d[r] = tokens (2r, 2r+1) concatenated).

Per core (b = core//4, n = core%4), channel-major [C, S] throughout:
  ctok = cfT + posT ; ctmp = compT + posT
  qT = Wq^T ctok ; kT = Wkv[:, :C]^T ctmp ; v = ctmp^T Wkv[:, C:]
  per head h: P = exp(scale k_h^T q); o_h = (v_h^T P) / Z   (Z via ones col)
  s_tok = packed^T Wproj                                     (token-major)
  const (token-quarter n, full scale): s_cq = ctokQ^T Wproj + bproj
  out_p = WconvT[:C]^T s2d + WconvT[quarter]^T s2d_cq + bconv/4
  out_cf = WconvT[C:, out-quarter]^T cf_raw                  (host-placed)
Host sums the 4 component partials per batch.  The affine const terms are
distributed: each core owns token-quarter n of the ctok-proj path and
out-channel quarter n of the cf-conv, plus bconv/4 — no gated-zero work.

Schedule: attention is ACT(exp)-bound at ~1.3us/kt, so only v and the
(kT, qT) pair for head 0/1 are computed up front; the remaining k/q groups,
the const-proj quarter, and the cf-conv quarter are emitted INTO the head
loop to fill PE slack under the exp stream.  P/V run in bf16; other matmuls
fp32r.  Heads packed two-per-tile so proj runs at 128 contraction.
"""
import sys

sys.path.insert(0, "/opt/trn_rl_repo")

import numpy as np

N_CORES = 8
B, C, H, W = 2, 512, 32, 32
S = H * W  # 1024
NH, HD = 8, 64
SCALE = HD ** -0.5

_CACHE = {}


def _img(x, cols):
    """[512, cols] matrix -> its [128, 4*cols] SBUF image (4 row-blocks
    side by side)."""
    return np.ascontiguousarray(
        x.reshape(4, 128, cols).transpose(1, 0, 2).reshape(128, 4 * cols),
        dtype=np.float32)


def _build():
    if "nc" in _CACHE:
        return _CACHE["nc"]
    from contextlib import ExitStack

    import concourse.bacc as bacc
    import concourse.mybir as mybir
    import concourse.tile as tile

    f32 = mybir.dt.float32
    f32r = mybir.dt.float32r
    bf16 = mybir.dt.bfloat16
    EXP = mybir.ActivationFunctionType.Exp

    nc = bacc.Bacc("TRN2", target_bir_lowering=False, debug=False,
                   num_devices=N_CORES)

    din = lambda n, s: nc.dram_tensor(n, s, mybir.dt.float32r,
                                      kind="ExternalInput").ap()
    pos_d = din("pos", [128, 4096])      # posT image
    cmp_d = din("cmp", [128, 4096])      # compT image
    cft_d = din("cft", [128, 4096])      # cfT image
    cfr_d = din("cfr", [128, 4096])      # raw content_feat[b] image
    wk_d = din("wk", [128, 2048])        # Wkv[:, :C] image
    wv_d = din("wv", [128, 2048])        # Wkv[:, C:] image
    wq_d = din("wq", [128, 2048])        # Wq image
    wproj_d = din("wproj", [128, 2048])  # Wproj image
    wcvs_d = din("wcvs", [128, 2048])    # WconvT[:C] image
    wcvcq_d = din("wcvcq", [128, 512])   # WconvT[C:, out-quarter] image
    wcvsq_d = din("wcvsq", [128, 512])   # WconvT[128n:128(n+1), :]
    cftq_d = din("cftq", [128, 1024])    # cfT[:, token-quarter] image
    posq_d = din("posq", [128, 1024])    # posT[:, token-quarter] image
    bias_d = din("bias2", [1, 1024])     # [bproj, bconv/4]
    out_p = nc.dram_tensor("out_p", [C, S], f32, kind="ExternalOutput").ap()
    out_cf = nc.dram_tensor("out_cf", [128, S], f32,
                            kind="ExternalOutput").ap()

    with tile.TileContext(nc) as tc, ExitStack() as ctx:
        main = ctx.enter_context(tc.tile_pool(name="main", bufs=1))

        ones32 = main.tile([1, 512], f32, tag="ones32")
        nc.gpsimd.memset(ones32[:], 1.0)
        ones = main.tile([1, 512], f32r, tag="ones")
        nc.vector.tensor_copy(ones[:], ones32[:])

        # ---- front-critical DMAs: v path first ----
        pos_sb = [main.tile([128, 1024], f32r, tag=f"pos{j}", name=f"pos{j}")
                  for j in range(4)]
        cmp_sb = [main.tile([128, 1024], f32r, tag=f"cmp{j}", name=f"cmp{j}")
                  for j in range(4)]
        cft_sb = [main.tile([128, 1024], f32r, tag=f"cft{j}", name=f"cft{j}")
                  for j in range(4)]
        wv_sb = main.tile([128, 2048], f32r, tag="wv")
        wk_sb = main.tile([128, 2048], f32r, tag="wk")
        wq_sb = main.tile([128, 2048], f32r, tag="wq")
        ctok = [main.tile([128, S], f32r, tag=f"ctk{j}", name=f"ctok{j}")
                for j in range(4)]
        ctmp = [main.tile([128, S], f32r, tag=f"ct{j}", name=f"ctmp{j}")
                for j in range(4)]
        nc.sync.dma_start(wv_sb[:, 0:1024], wv_d[:, 0:1024])
        for j in range(4):
            nc.sync.dma_start(pos_sb[j][:], pos_d[:, 1024 * j:1024 * (j + 1)])
            nc.sync.dma_start(cmp_sb[j][:], cmp_d[:, 1024 * j:1024 * (j + 1)])
            if j == 0:
                nc.sync.dma_start(wv_sb[:, 1024:2048], wv_d[:, 1024:2048])
            if j == 1:
                nc.sync.dma_start(wk_sb[:, 0:1024], wk_d[:, 0:1024])
            if j == 2:
                nc.sync.dma_start(wk_sb[:, 1024:2048], wk_d[:, 1024:2048])
            eng = nc.vector if j % 2 == 0 else nc.gpsimd
            eng.tensor_add(ctmp[j][:], cmp_sb[j][:], pos_sb[j][:])
        nc.sync.dma_start(wq_sb[:], wq_d[:])
        for j in range(4):
            nc.sync.dma_start(cft_sb[j][:], cft_d[:, 1024 * j:1024 * (j + 1)])
            eng = nc.gpsimd if j % 2 == 0 else nc.vector
            eng.tensor_add(ctok[j][:], cft_sb[j][:], pos_sb[j][:])

        # ---- late weights / const-path inputs ----
        cfr_sb = [main.tile([128, 1024], f32r, tag=f"cfr{j}", name=f"cfr{j}")
                  for j in range(4)]
        wproj_sb = main.tile([128, 2048], f32r, tag="wp")
        wcvs_sb = main.tile([128, 2048], f32r, tag="wcs")
        wcvcq_sb = main.tile([128, 512], f32r, tag="wcc")
        wcvsq_sb = main.tile([128, 512], f32r, tag="wcsq")
        cftq_sb = main.tile([128, 1024], f32r, tag="cftq")
        posq_sb = main.tile([128, 1024], f32r, tag="posq")
        bias_sb = main.tile([1, 1024], f32r, tag="bias")
        nc.sync.dma_start(wproj_sb[:], wproj_d[:])
        for j in range(4):
            nc.sync.dma_start(cfr_sb[j][:], cfr_d[:, 1024 * j:1024 * (j + 1)])
        nc.sync.dma_start(wcvcq_sb[:], wcvcq_d[:])
        nc.sync.dma_start(cftq_sb[:], cftq_d[:])
        nc.sync.dma_start(posq_sb[:], posq_d[:])
        nc.sync.dma_start(bias_sb[:], bias_d[:])
        nc.sync.dma_start(wcvs_sb[:], wcvs_d[:])
        nc.sync.dma_start(wcvsq_sb[:], wcvsq_d[:])

        # norm scratch + late tiles (several ride dead early slots)
        zraw = main.tile([1, S], f32, tag="zraw")
        zs2 = main.tile([1, S], f32, tag="zs2")
        zinv = main.tile([1, S], f32, tag="zinv")
        zbc = main.tile([128, S], f32, tag="zbc")
        ocf_sb = main.tile([128, S], f32, tag="cfr0", name="ocf")
        ctokq = main.tile([128, 1024], f32r, tag="wq", name="ctokq")
        packed = [main.tile([128, S], f32r,
                            tag=(f"cfr{j}" if j else "zzpk0"),
                            name=f"pk{j}") for j in range(4)]
        outp = [main.tile([128, S], f32, tag=f"ctk{j}", name=f"op{j}")
                for j in range(4)]
        kT = [main.tile([128, S], f32r, tag=f"cft{j}", name=f"kT{j}")
              for j in range(4)]
        qT = [main.tile([128, S], f32r, tag=f"cmp{j}", name=f"qT{j}")
              for j in range(4)]
        v_sb = [main.tile([128, 1040], bf16, tag=f"v{t}", name=f"v{t}")
                for t in range(4)]

        def kq_group(psum, j):
            """k and q projection for head-pair j (kT copy DVE, qT ACT)."""
            for qc in range(2):
                acc = psum.tile([128, 512], f32, tag="mm")
                for k in range(4):
                    nc.tensor.matmul(
                        acc[:],
                        wk_sb[:, 512 * k + 128 * j:512 * k + 128 * (j + 1)],
                        ctmp[k][:, 512 * qc:512 * (qc + 1)],
                        start=(k == 0), stop=(k == 3))
                nc.vector.tensor_copy(kT[j][:, 512 * qc:512 * (qc + 1)],
                                      acc[:])
                acc2 = psum.tile([128, 512], f32, tag="mm")
                for k in range(4):
                    nc.tensor.matmul(
                        acc2[:],
                        wq_sb[:, 512 * k + 128 * j:512 * k + 128 * (j + 1)],
                        ctok[k][:, 512 * qc:512 * (qc + 1)],
                        start=(k == 0), stop=(k == 3))
                nc.vector.tensor_copy(qT[j][:, 512 * qc:512 * (qc + 1)],
                                      acc2[:])

        with tc.tile_pool(name="psA", bufs=2, space="PSUM") as ps:
            # ---- v: [ki, hd] blocks per (kt, head) + ones col, bf16 ----
            for t in range(4):
                nc.gpsimd.memset(v_sb[t][:], 1.0)
            for kt in range(8):
                acc = ps.tile([128, 512], f32, tag="mm")
                for k in range(4):
                    nc.tensor.matmul(acc[:],
                                     ctmp[k][:, 128 * kt:128 * (kt + 1)],
                                     wv_sb[:, 512 * k:512 * (k + 1)],
                                     start=(k == 0), stop=(k == 3))
                dst = v_sb[kt // 2][:, 520 * (kt % 2):520 * (kt % 2) + 520]
                nc.scalar.copy(
                    dst.rearrange("p (m c) -> p m c", m=8)[:, :, 0:64],
                    acc[:].rearrange("p (m c) -> p m c", m=8))
            kq_group(ps, 0)

        # ---- attention, with remaining work streamed into PE slack ----
        with tc.tile_pool(name="psS", bufs=2, space="PSUM") as psS, \
             tc.tile_pool(name="psO", bufs=1, space="PSUM") as psO, \
             tc.tile_pool(name="psT", bufs=2, space="PSUM") as psT:
            ptp = [main.tile([128, S], bf16, tag=f"pt{t}", name=f"pt{t}")
                   for t in range(8)]
            s_cq = [main.tile([128, 512], f32r, tag=f"ct{i}", name=f"scq{i}")
                    for i in range(2)]
            s2d_cq = main.tile([128, S], f32r, tag="ct2", name="s2dcq")
            for h in range(NH):
                j, row = h // 2, 64 * (h % 2)
                o_ps = psO.tile([65, S], f32, tag="o")
                for kt in range(8):
                    sc = psS.tile([128, S], f32, tag="sc")
                    for qc in range(2):
                        nc.tensor.matmul(
                            sc[:, 512 * qc:512 * (qc + 1)],
                            kT[j][row:row + 64, 128 * kt:128 * (kt + 1)],
                            qT[j][row:row + 64, 512 * qc:512 * (qc + 1)],
                            start=True, stop=True)
                    pt = ptp[kt]
                    nc.scalar.activation(pt[:], sc[:], EXP, scale=SCALE)
                    vsl = v_sb[kt // 2][:, 520 * (kt % 2) + 65 * h:
                                        520 * (kt % 2) + 65 * h + 65]
                    for qc in range(2):
                        nc.tensor.matmul(
                            o_ps[:, 512 * qc:512 * (qc + 1)], vsl,
                            pt[:, 512 * qc:512 * (qc + 1)],
                            start=(kt == 0), stop=(kt == 7))
                # normalization: Z row -> recip -> broadcast -> scale
                nc.vector.tensor_copy(zraw[0:1, :], o_ps[64:65, :])
                nc.vector.reciprocal_approx_accurate(
                    zinv[0:1, :], zraw[0:1, :], zs2[0:1, :])
                nc.gpsimd.partition_broadcast(zbc[:], zinv[0:1, :])
                rows = slice(64 * (h % 2), 64 * (h % 2) + 64)
                nc.vector.tensor_copy(packed[j][rows, :], o_ps[0:64, :])
                nc.gpsimd.tensor_mul(packed[j][rows, :],
                                     packed[j][rows, :], zbc[rows, :])
                # stream the remaining independent work into PE slack
                if h < 3:
                    kq_group(psT, h + 1)
                elif h == 3:  # const-proj token-quarter
                    nc.gpsimd.tensor_add(ctokq[:], cftq_sb[:], posq_sb[:])
                    for i in range(2):
                        acc = psT.tile([128, 512], f32, tag="mm")
                        nc.tensor.matmul(acc[:], ones[0:1, 0:128],
                                         bias_sb[0:1, 0:512],
                                         start=True, stop=False)
                        for a in range(4):
                            nc.tensor.matmul(
                                acc[:],
                                ctokq[:, 256 * a + 128 * i:
                                      256 * a + 128 * (i + 1)],
                                wproj_sb[:, 512 * a:512 * (a + 1)],
                                start=False, stop=(a == 3))
                        nc.vector.tensor_copy(s_cq[i][:], acc[:])
                    for g in range(2):
                        for sh in range(2):
                            nc.sync.dma_start(
                                s2d_cq[64 * sh:64 * sh + 64,
                                       512 * g:512 * g + 512],
                                s_cq[sh][g:128:2, :])
                elif h == 4:  # cf-conv output-channel quarter
                    for half in range(2):
                        acc = psT.tile([128, 512], f32, tag="mm")
                        for k in range(4):
                            nc.tensor.matmul(
                                acc[:], wcvcq_sb[:, 128 * k:128 * (k + 1)],
                                cfr_sb[k][:, 512 * half:512 * (half + 1)],
                                start=(k == 0), stop=(k == 3))
                        nc.vector.tensor_copy(
                            ocf_sb[:, 512 * half:512 * (half + 1)], acc[:])
                    nc.sync.dma_start(out_cf[:, :], ocf_sb[:])

            # ---- proj (token-major, 4 mms/tile) -> s2d re-pair -> conv ----
            s_sb = [main.tile([128, 512], f32r, tag=f"cmp{i % 4}",
                              name=f"s{i}") for i in range(8)]
            s2d = [main.tile([128, S], f32r, tag=f"cft{i}", name=f"s2d{i}")
                   for i in range(4)]
            for i in range(8):
                acc = psT.tile([128, 512], f32, tag="mm")
                for jj in range(4):
                    nc.tensor.matmul(acc[:],
                                     packed[jj][:, 128 * i:128 * (i + 1)],
                                     wproj_sb[:, 512 * jj:512 * (jj + 1)],
                                     start=(jj == 0), stop=(jj == 3))
                eng = nc.vector if i % 2 == 0 else nc.scalar
                if eng is nc.scalar:
                    eng.copy(s_sb[i][:], acc[:])
                else:
                    eng.tensor_copy(s_sb[i][:], acc[:])
                if i % 2 == 1:
                    ii = i // 2
                    for g in range(2):
                        for sh in range(2):
                            src = s_sb[2 * ii + sh]
                            nc.sync.dma_start(
                                s2d[ii][64 * sh:64 * sh + 64,
                                        512 * g:512 * g + 512],
                                src[g:128:2, :])
            for oc in range(4):
                for half in range(2):
                    acc = psT.tile([128, 512], f32, tag="mm")
                    nc.tensor.matmul(
                        acc[:],
                        bias_sb[0:1, 512 + 128 * oc:512 + 128 * (oc + 1)],
                        ones[0:1, :], start=True, stop=False)
                    for r in range(4):
                        nc.tensor.matmul(
                            acc[:],
                            wcvs_sb[:, 512 * r + 128 * oc:
                                    512 * r + 128 * (oc + 1)],
                            s2d[r][:, 512 * half:512 * (half + 1)],
                            start=False, stop=False)
                    nc.tensor.matmul(
                        acc[:], wcvsq_sb[:, 128 * oc:128 * (oc + 1)],
                        s2d_cq[:, 512 * half:512 * (half + 1)],
                        start=False, stop=True)
                    eng = nc.vector if (oc + half) % 2 == 0 else nc.scalar
                    if eng is nc.scalar:
                        eng.copy(outp[oc][:, 512 * half:512 * (half + 1)],
                                 acc[:])
                    else:
                        eng.tensor_copy(
                            outp[oc][:, 512 * half:512 * (half + 1)], acc[:])
                    nc.sync.dma_start(
                        out_p[128 * oc:128 * (oc + 1),
                              512 * half:512 * (half + 1)],
                        outp[oc][:, 512 * half:512 * (half + 1)])

    nc.compile()
    _CACHE["nc"] = nc
    return nc


def _shard_inputs(content_feat, components, pos_emb, Wq, Wkv, Wproj, bproj,
                  Wconv, bconv):
    f = np.float32
    posT = pos_emb.reshape(S, C).T.astype(f)
    pos_img = _img(posT, S)
    wconvT = Wconv.T.astype(f)                        # [2C, C]
    wk_img = _img(np.ascontiguousarray(Wkv[:, :C]), C)
    wv_img = _img(np.ascontiguousarray(Wkv[:, C:]), C)
    wq_img = _img(Wq, C)
    wproj_img = _img(Wproj, C)
    wcvs_img = _img(np.ascontiguousarray(wconvT[:C]), C)
    bias2 = np.ascontiguousarray(
        np.concatenate([bproj, bconv / 4]).reshape(1, 1024), dtype=f)
    in_maps = []
    for core in range(N_CORES):
        b, n = core // 4, core % 4
        sl = slice(128 * n, 128 * (n + 1))
        tq = slice(256 * n, 256 * (n + 1))
        cfT = np.ascontiguousarray(content_feat[b].reshape(S, C).T)
        in_maps.append({
            "pos": pos_img,
            "cmp": _img(components[n, b].reshape(S, C).T, S),
            "cft": _img(cfT, S),
            "cfr": _img(content_feat[b].reshape(C, S), S),
            "wk": wk_img,
            "wv": wv_img,
            "wq": wq_img,
            "wproj": wproj_img,
            "wcvs": wcvs_img,
            "wcvcq": _img(np.ascontiguousarray(wconvT[C:, sl]), 128),
            "wcvsq": np.ascontiguousarray(wconvT[sl], dtype=f),
            "cftq": _img(np.ascontiguousarray(cfT[:, tq]), 256),
            "posq": _img(np.ascontiguousarray(posT[:, tq]), 256),
            "bias2": bias2,
        })
    return in_maps


def _run(trace=False, **inputs):
    from concourse.bass_utils import run_bass_kernel_spmd

    nc = _build()
    in_maps = _shard_inputs(**inputs)
    res = run_bass_kernel_spmd(nc, in_maps, list(range(N_CORES)), trace=trace)
    full = np.empty((B, C, S), dtype=np.float32)
    for b in range(B):
        acc = sum(res.results[4 * b + n]["out_p"] for n in range(4))
        for n in range(4):
            acc[128 * n:128 * (n + 1)] += res.results[4 * b + n]["out_cf"]
        full[b] = acc
    return full.reshape(B, C, H, W).astype(np.float32), res


def kernel(**inputs):
    out, _ = _run(trace=False, **inputs)
    return out
